# revision 31
# baseline (speedup 1.0000x reference)
"""Multi-head attention (RoPE, causal) on 8 TRN2 NeuronCores.

Sharding: DP2 x TP4. Core c handles batch b = c//4 and heads
H_c = {4*(c%4) .. 4*(c%4)+3}. Inside each batch group of 4 cores the
attention outputs are exchanged with an AllToAll (bf16, q-sliced), after
which every core computes the final out-projection for its 512-row
q-slice with the full head dimension locally. No reduction collective is
needed; the host-side unshard is a pure concatenation.

Device math (validated in numpy to ~7e-6 rel err vs the jax reference):
  - x^T built via PE transposes; QKV projection with x^T so Q,K come out
    pre-transposed ([head_dim, seq]) for the scores matmul; V natural.
  - RoPE rotate-every-two as a matmul with a constant +-1 permutation
    matrix, then cos/sin multiply-adds on DVE.
  - softmax without max subtraction (scores ~ N(0,1), exp cannot
    overflow); denominator via an appended ones-row in V; division by
    the denominator folded into the PSUM->SBUF copy of the attention
    output, with the reciprocal broadcast across partitions by a K=1
    ones matmul.
  - causal masking at block granularity (strictly-upper blocks skipped)
    plus affine_select on the 4 diagonal blocks per q-chunk.
  - matmuls in float32r (full-rate fp32 path of the PE).
"""

import sys

for _p in ("/opt/trn_rl_repo",):
    if _p not in sys.path:
        sys.path.insert(0, _p)

import numpy as np
import ml_dtypes

from concourse import bacc, bass, mybir, tile
from concourse.bass_utils import run_bass_kernel_spmd
from concourse.masks import make_identity

F32 = mybir.dt.float32
F32R = mybir.dt.float32r
BF16 = mybir.dt.bfloat16

D, H, HD, S, B = 1024, 16, 64, 2048, 2
HPC = 4          # heads per core
NP = 2           # head pairs per core
QC = 512         # q-chunk size
KB = 128         # k-block size
NQC = S // QC    # 4
NKB = S // KB    # 16
GS = 4           # cores per batch group
NC = 8           # total cores; the AllToAll spans all 8
SLC = S // NC    # 256 rows of final output per core (for BOTH batches)

Copy = mybir.ActivationFunctionType.Copy
Ident = mybir.ActivationFunctionType.Identity
Exp = mybir.ActivationFunctionType.Exp


def _host_constants():
    pos = np.arange(S, dtype=np.float64)
    inv_freq = 1.0 / (10000.0 ** (np.arange(0, HD, 2, dtype=np.float64) / HD))
    freqs = np.outer(pos, inv_freq)
    cosT = np.repeat(np.cos(freqs), 2, axis=1).T.astype(np.float32)  # [64, S]
    sinT = np.repeat(np.sin(freqs), 2, axis=1).T.astype(np.float32)
    # pair-stacked: same table on both 64-partition halves
    cosT = np.concatenate([cosT, cosT], axis=0)  # [128, S]
    sinT = np.concatenate([sinT, sinT], axis=0)
    perm = np.zeros((128, 128), dtype=np.float32)
    for base in (0, 64):
        for i in range(32):
            perm[base + 2 * i + 1, base + 2 * i] = -1.0
            perm[base + 2 * i, base + 2 * i + 1] = 1.0
    return cosT, sinT, perm


def build_program():
    cosT, sinT, perm_np = _host_constants()

    nc = bacc.Bacc(None, target_bir_lowering=False)

    # --- I/O ---------------------------------------------------------
    xb = nc.declare_dram_parameter("xb", [S, D], F32, isOutput=False)
    wq = nc.declare_dram_parameter("wq", [D, 256], F32R, isOutput=False)
    wk = nc.declare_dram_parameter("wk", [D, 256], F32R, isOutput=False)
    wv = nc.declare_dram_parameter("wv", [D, 260], F32R, isOutput=False)
    bq = nc.declare_dram_parameter("bq", [NP, 128], F32, isOutput=False)
    bk = nc.declare_dram_parameter("bk", [NP, 128], F32, isOutput=False)
    bv = nc.declare_dram_parameter("bv", [1, 260], F32R, isOutput=False)
    ones = nc.declare_dram_parameter("ones", [128, 128], F32R, isOutput=False)
    perm = nc.declare_dram_parameter("perm", [128, 128], F32R, isOutput=False)
    wout = nc.declare_dram_parameter("wout", [D, D], BF16, isOutput=False)
    bout = nc.declare_dram_parameter("bout", [1, D], BF16, isOutput=False)
    out = nc.declare_dram_parameter("out_s", [B, SLC, D], F32, isOutput=True)

    cos_c = nc.inline_tensor(cosT, name="cos_c")
    sin_c = nc.inline_tensor(sinT, name="sin_c")

    with tile.TileContext(nc) as tc:
        with (
            tc.tile_pool(name="persist", bufs=1) as pp,
            tc.tile_pool(name="dram", bufs=1, space="DRAM") as dp,
        ):
            # --- constants / weights into SBUF ---------------------------
            ident = pp.tile([128, 128], F32)
            make_identity(nc, ident)
            perm_s = pp.tile([128, 128], F32R)
            nc.sync.dma_start(out=perm_s[:], in_=perm[:])
            cos_s = pp.tile([128, S], F32)
            sin_s = pp.tile([128, S], F32)
            nc.sync.dma_start(out=cos_s[:], in_=cos_c[:])
            nc.sync.dma_start(out=sin_s[:], in_=sin_c[:])
            ones_f = pp.tile([128, 128], F32R)
            nc.sync.dma_start(out=ones_f[:], in_=ones[:])
            ones_b = pp.tile([1, 128], BF16)
            nc.vector.memset(ones_b[:], 1.0)

            wq_s = pp.tile([128, 8, 256], F32R)
            wk_s = pp.tile([128, 8, 256], F32R)
            wv_s = pp.tile([128, 8, 260], F32R)
            nc.sync.dma_start(out=wq_s[:], in_=wq.rearrange("(c p) n -> p c n", p=128))
            nc.sync.dma_start(out=wk_s[:], in_=wk.rearrange("(c p) n -> p c n", p=128))
            nc.sync.dma_start(out=wv_s[:], in_=wv.rearrange("(c p) n -> p c n", p=128))
            bq_s = pp.tile([128, NP], F32)
            bk_s = pp.tile([128, NP], F32)
            bv_s = pp.tile([1, 260], F32R)
            for p in range(NP):
                nc.sync.dma_start(out=bq_s[:, p : p + 1], in_=bq[p][:, None])
                nc.sync.dma_start(out=bk_s[:, p : p + 1], in_=bk[p][:, None])
            nc.sync.dma_start(out=bv_s[:], in_=bv[:])

            # persistent activations
            qt = pp.tile([128, NP * S], F32R)   # rotated Q^T, pair-major
            kt = pp.tile([128, NP * S], F32R)   # rotated K^T
            vt = [pp.tile([128, HPC * 65], F32R, name=f"vt{i}") for i in range(NKB)]
            # attnT[p]: [64, 2*S] bf16 — within-pair head h at cols [S*h, S*(h+1))
            attnT = [pp.tile([64, NP * S], BF16, name=f"attnT{p}") for p in range(NP)]

            # DRAM bounce buffers for the per-pair AllToAll (8 ranks; rank r
            # receives every core's attnT columns [SLC*r, SLC*(r+1)) )
            cc_in = [
                dp.tile([NC, 128, SLC], BF16, name=f"cc_in{p}") for p in range(NP)
            ]
            cc_out = [
                dp.tile([NC, 128, SLC], BF16, name=f"cc_out{p}") for p in range(NP)
            ]

            # =============================================================
            # Phase A: x^T, QKV projection, RoPE
            # =============================================================
            with (
                tc.tile_pool(name="xt_pool", bufs=1) as xtp,
                tc.tile_pool(name="x_pool", bufs=2) as xp,
                tc.tile_pool(name="qkraw", bufs=3) as rawp,
                tc.tile_pool(name="tp_psum", bufs=2, space="PSUM") as tpp,
                tc.tile_pool(name="pj_psum", bufs=3, space="PSUM") as pjp,
                tc.tile_pool(name="rp_psum", bufs=2, space="PSUM") as rpp,
            ):
                # x^T: [128, 8*2048], chunk dc at cols [2048*dc, +2048)
                xt = xtp.tile([128, 8 * S], F32R)
                for sb in range(16):
                    x_t = xp.tile([128, D], F32, tag="x")
                    nc.sync.dma_start(out=x_t[:], in_=xb[128 * sb : 128 * sb + 128, :])
                    for dg in range(2):  # two PSUM banks of 4 transposes
                        ps = tpp.tile([128, 512], F32, tag="tp")
                        for i in range(4):
                            dc = 4 * dg + i
                            nc.tensor.transpose(
                                ps[:, 128 * i : 128 * i + 128],
                                x_t[:, 128 * dc : 128 * dc + 128],
                                ident[:],
                            )
                        # strided copy into the 4 dc-chunks of xt
                        dst = xt.rearrange("p (c s) -> p c s", s=S)[
                            :, 4 * dg : 4 * dg + 4, 128 * sb : 128 * sb + 128
                        ]
                        src = ps.rearrange("p (c s) -> p c s", s=128)
                        nc.vector.tensor_copy(dst, src)

                # QKV projection + RoPE, chunk-wise.
                # Q^T/K^T: [dq(128 per pair), s] ; rot = raw*cos + (P.T@raw)*sin
                for p in range(NP):
                    for sc in range(NQC):
                        ssl = slice(QC * sc, QC * sc + QC)
                        for w_s, b_s, rot in (
                            (wq_s, bq_s, qt),
                            (wk_s, bk_s, kt),
                        ):
                            ps = pjp.tile([128, 512], F32, tag="pj")
                            for c in range(8):
                                nc.tensor.matmul(
                                    ps[:],
                                    w_s[:, c, 128 * p : 128 * p + 128],
                                    xt[:, S * c + QC * sc : S * c + QC * sc + QC],
                                    start=(c == 0),
                                    stop=(c == 7),
                                )
                            raw = rawp.tile([128, 512], F32R, tag="raw")
                            nc.scalar.activation(
                                raw[:], ps[:], Ident, bias=b_s[:, p : p + 1]
                            )
                            pr = rpp.tile([128, 512], F32, tag="rp")
                            nc.tensor.matmul(
                                pr[:],
                                perm_s[:],
                                raw[:],
                                start=True,
                                stop=True,
                            )
                            dst = rot[:, S * p + QC * sc : S * p + QC * sc + QC]
                            nc.vector.tensor_mul(dst, raw[:], cos_s[:, ssl])
                            nc.vector.tensor_mul(pr[:], pr[:], sin_s[:, ssl])
                            nc.vector.tensor_add(dst, dst, pr[:])

                # V natural [s, 4*65] with a ones column per head slot
                for sb in range(16):
                    ps = pjp.tile([128, 260], F32, tag="pj")
                    for c in range(8):
                        nc.tensor.matmul(
                            ps[:],
                            xt[:, S * c + 128 * sb : S * c + 128 * sb + 128],
                            wv_s[:, c, :],
                            start=(c == 0),
                            stop=False,
                        )
                    nc.tensor.matmul(
                        ps[:],
                        ones_f[0:1, 0:128],
                        bv_s[:],
                        start=False,
                        stop=True,
                    )
                    nc.vector.tensor_copy(vt[sb][:], ps[:])

            # =============================================================
            # Phase B: attention per (pair, q-chunk), heads interleaved
            # =============================================================
            with (
                tc.tile_pool(name="p_pool", bufs=12) as ppool,
                tc.tile_pool(name="recip", bufs=4) as rcp,
                tc.tile_pool(name="sc_psum", bufs=3, space="PSUM") as scp,
                tc.tile_pool(name="av_psum", bufs=3, space="PSUM") as avp,
                tc.tile_pool(name="bc_psum", bufs=2, space="PSUM") as bcp,
            ):
                for p in range(NP):
                    for qc in range(NQC):
                        qsl = slice(QC * qc, QC * qc + QC)
                        nkb_q = 4 * qc + 4
                        av = [
                            avp.tile([128, 512], F32, tag="av", name=f"av{_h}")
                            for _h in range(2)
                        ]
                        for kb in range(nkb_q):
                            pt = []
                            for h in range(2):
                                hsl = slice(64 * h, 64 * h + 64)
                                sc_ps = scp.tile([128, 512], F32, tag="sc")
                                nc.tensor.matmul(
                                    sc_ps[:],
                                    kt[hsl, S * p + KB * kb : S * p + KB * kb + KB],
                                    qt[hsl, S * p + QC * qc : S * p + QC * qc + QC],
                                    start=True,
                                    stop=True,
                                )
                                p_t = ppool.tile([128, 512], F32R, tag="p")
                                nc.scalar.activation(
                                    p_t[:], sc_ps[:], Exp, scale=float(HD**-0.5)
                                )
                                mrel = kb - 4 * qc
                                if mrel >= 0:
                                    # zero where q < k: keep when c - r - 128*mrel >= 0
                                    nc.gpsimd.affine_select(
                                        out=p_t[:],
                                        in_=p_t[:],
                                        compare_op=mybir.AluOpType.is_ge,
                                        fill=0.0,
                                        base=-128 * mrel,
                                        pattern=[[1, 512]],
                                        channel_multiplier=-1,
                                    )
                                pt.append(p_t)
                            for h in range(2):
                                hl = 2 * p + h
                                nc.tensor.matmul(
                                    av[h][0:65, :],
                                    vt[kb][:, 65 * hl : 65 * hl + 65],
                                    pt[h][:],
                                    start=(kb == 0),
                                    stop=(kb == nkb_q - 1),
                                )
                        for h in range(2):
                            rc = rcp.tile([65, 512], F32R, tag="rc")
                            with nc.allow_low_precision(
                                reason="softmax denom reciprocal feeds an "
                                "fp32r matmul; ~19-bit mantissa is plenty"
                            ):
                                nc.vector.reciprocal(rc[64:65, :], av[h][64:65, :])
                            bc = bcp.tile([64, 512], F32, tag="bc")
                            nc.tensor.matmul(
                                bc[:],
                                ones_f[64:65, 0:64],
                                rc[64:65, :],
                                start=True,
                                stop=True,
                            )
                            avs = rcp.tile([64, 512], F32, tag="avs")
                            nc.vector.tensor_copy(avs[:], av[h][0:64, :])
                            nc.vector.tensor_mul(
                                attnT[p][:, S * h + QC * qc : S * h + QC * qc + QC],
                                avs[:],
                                bc[:],
                            )
                    # exchange this pair's attention output (overlaps next pair)
                    for h in range(2):
                        nc.sync.dma_start(
                            out=cc_in[p].rearrange("g p q -> p g q")[
                                64 * h : 64 * h + 64
                            ],
                            in_=attnT[p][
                                :, S * h : S * h + S
                            ].rearrange("p (g q) -> p g q", g=NC),
                        )
                    nc.gpsimd.collective_compute(
                        "AllToAll",
                        mybir.AluOpType.bypass,
                        ins=[cc_in[p].opt()],
                        outs=[cc_out[p].opt()],
                        replica_groups=[[0, 1, 2, 3, 4, 5, 6, 7]],
                    )

            # =============================================================
            # Phase C: gathered attn^T -> out projection for my q-slice
            # =============================================================
            with (
                tc.tile_pool(name="af_pool", bufs=1) as afp,
                tc.tile_pool(name="wo_pool", bufs=1) as wop,
                tc.tile_pool(name="out_sb", bufs=4) as osp,
                tc.tile_pool(name="op_psum", bufs=2, space="PSUM") as opp,
            ):
                wo_s = wop.tile([128, 8, D], BF16)
                nc.sync.dma_start(
                    out=wo_s[:], in_=wout.rearrange("(c p) n -> p c n", p=128)
                )
                bo_s = wop.tile([1, D], BF16)
                nc.sync.dma_start(out=bo_s[:], in_=bout[:])

                # af[b2][k]: head-dim chunk k (rows [128k, 128k+128) of attn
                # for batch b2) over my SLC q-rows.  Source core of chunk k is
                # 4*b2 + k//2, pair k%2.
                af = [
                    [afp.tile([128, SLC], BF16, name=f"af{b2}_{k}") for k in range(8)]
                    for b2 in range(B)
                ]
                for p in range(NP):
                    for src in range(NC):
                        b2, g = src // 4, src % 4
                        nc.sync.dma_start(
                            out=af[b2][2 * g + p][:], in_=cc_out[p][src]
                        )

                for b2 in range(B):
                    for sb in range(SLC // 128):
                        for nc2 in range(2):
                            nsl = slice(512 * nc2, 512 * nc2 + 512)
                            ps = opp.tile([128, 512], F32, tag="op")
                            nc.tensor.matmul(
                                ps[:],
                                ones_b[:, 0:128],
                                bo_s[:, nsl],
                                start=True,
                                stop=False,
                            )
                            for k in range(8):
                                nc.tensor.matmul(
                                    ps[:],
                                    af[b2][k][:, 128 * sb : 128 * sb + 128],
                                    wo_s[:, k, nsl],
                                    start=False,
                                    stop=(k == 7),
                                )
                            o_t = osp.tile([128, 512], F32, tag="o")
                            nc.vector.tensor_copy(o_t[:], ps[:])
                            nc.sync.dma_start(
                                out=out[b2, 128 * sb : 128 * sb + 128, nsl],
                                in_=o_t[:],
                            )
    nc.finalize()
    return nc


_PROGRAM = None


def _get_program():
    global _PROGRAM
    if _PROGRAM is None:
        _PROGRAM = build_program()
    return _PROGRAM


def make_in_maps(x, Wqkv, bqkv, Wout, bout):
    x = np.asarray(x, dtype=np.float32)
    Wqkv = np.asarray(Wqkv, dtype=np.float32)
    bqkv = np.asarray(bqkv, dtype=np.float32)
    Wout = np.asarray(Wout, dtype=np.float32)
    bout = np.asarray(bout, dtype=np.float32)

    wout_bf = Wout.astype(ml_dtypes.bfloat16)
    bout_bf = bout.reshape(1, D).astype(ml_dtypes.bfloat16)
    _, _, perm_np = _host_constants()
    ones_np = np.ones((128, 128), dtype=np.float32)
    in_maps = []
    for c in range(8):
        b, g = c // 4, c % 4
        cols = slice(64 * HPC * g, 64 * HPC * (g + 1))  # this core's head dims
        # V weights augmented with a zero column per head slot; the matching
        # bias element is 1.0, so V tiles come out as [v(64) | 1] per head.
        wv_aug = np.zeros((D, 65 * HPC), dtype=np.float32)
        bv_aug = np.zeros((1, 65 * HPC), dtype=np.float32)
        wv_c = Wqkv[:, 2 * D :][:, cols]
        bv_c = bqkv[2 * D :][cols]
        for h in range(HPC):
            wv_aug[:, 65 * h : 65 * h + 64] = wv_c[:, 64 * h : 64 * h + 64]
            bv_aug[0, 65 * h : 65 * h + 64] = bv_c[64 * h : 64 * h + 64]
            bv_aug[0, 65 * h + 64] = 1.0
        in_maps.append(
            {
                "xb": np.ascontiguousarray(x[:, b, :]),
                "wq": np.ascontiguousarray(Wqkv[:, 0 * D :][:, cols]),
                "wk": np.ascontiguousarray(Wqkv[:, 1 * D :][:, cols]),
                "wv": wv_aug,
                "bq": np.ascontiguousarray(bqkv[0 * D :][cols].reshape(NP, 128)),
                "bk": np.ascontiguousarray(bqkv[1 * D :][cols].reshape(NP, 128)),
                "bv": bv_aug,
                "ones": ones_np,
                "perm": perm_np,
                "wout": wout_bf,
                "bout": bout_bf,
            }
        )
    return in_maps


def unshard(results):
    out = np.empty((S, B, D), dtype=np.float32)
    for r in range(8):
        for b2 in range(B):
            out[SLC * r : SLC * (r + 1), b2, :] = results[r]["out_s"][b2]
    return out


def kernel(x, Wqkv, bqkv, Wout, bout, **_kw):
    nc = _get_program()
    in_maps = make_in_maps(x, Wqkv, bqkv, Wout, bout)
    res = run_bass_kernel_spmd(nc, in_maps, list(range(8)))
    return unshard(res.results)


# revision 34
# speedup vs baseline: 1.1527x; 1.1527x over previous
"""Multi-head attention (RoPE, causal) on 8 TRN2 NeuronCores.

Sharding: DP2 x TP4. Core c handles batch b = c//4 and heads
H_c = {4*(c%4) .. 4*(c%4)+3}. Inside each batch group of 4 cores the
attention outputs are exchanged with an AllToAll (bf16, q-sliced), after
which every core computes the final out-projection for its 512-row
q-slice with the full head dimension locally. No reduction collective is
needed; the host-side unshard is a pure concatenation.

Device math (validated in numpy to ~7e-6 rel err vs the jax reference):
  - x^T built via PE transposes; QKV projection with x^T so Q,K come out
    pre-transposed ([head_dim, seq]) for the scores matmul; V natural.
  - RoPE rotate-every-two as a matmul with a constant +-1 permutation
    matrix, then cos/sin multiply-adds on DVE.
  - softmax without max subtraction (scores ~ N(0,1), exp cannot
    overflow); denominator via an appended ones-row in V; division by
    the denominator folded into the PSUM->SBUF copy of the attention
    output, with the reciprocal broadcast across partitions by a K=1
    ones matmul.
  - causal masking at block granularity (strictly-upper blocks skipped)
    plus affine_select on the 4 diagonal blocks per q-chunk.
  - matmuls in float32r (full-rate fp32 path of the PE).
"""

import sys

for _p in ("/opt/trn_rl_repo",):
    if _p not in sys.path:
        sys.path.insert(0, _p)

import numpy as np
import ml_dtypes

from concourse import bacc, bass, mybir, tile
from concourse.bass_utils import run_bass_kernel_spmd

F32 = mybir.dt.float32
F32R = mybir.dt.float32r
BF16 = mybir.dt.bfloat16

D, H, HD, S, B = 1024, 16, 64, 2048, 2
HPC = 4          # heads per core
NP = 2           # head pairs per core
QC = 512         # q-chunk size
KB = 128         # k-block size
NQC = S // QC    # 4
NKB = S // KB    # 16
GS = 4           # cores per batch group
NC = 8           # total cores; the AllToAll spans all 8
SLC = S // NC    # 256 rows of final output per core (for BOTH batches)

Copy = mybir.ActivationFunctionType.Copy
Ident = mybir.ActivationFunctionType.Identity
Exp = mybir.ActivationFunctionType.Exp


def _host_constants():
    pos = np.arange(S, dtype=np.float64)
    inv_freq = 1.0 / (10000.0 ** (np.arange(0, HD, 2, dtype=np.float64) / HD))
    freqs = np.outer(pos, inv_freq)
    cosT = np.repeat(np.cos(freqs), 2, axis=1).T.astype(np.float32)  # [64, S]
    sinT = np.repeat(np.sin(freqs), 2, axis=1).T.astype(np.float32)
    # pair-stacked: same table on both 64-partition halves
    cosT = np.concatenate([cosT, cosT], axis=0)  # [128, S]
    sinT = np.concatenate([sinT, sinT], axis=0)
    perm = np.zeros((128, 128), dtype=np.float32)
    for base in (0, 64):
        for i in range(32):
            perm[base + 2 * i + 1, base + 2 * i] = -1.0
            perm[base + 2 * i, base + 2 * i + 1] = 1.0
    return cosT, sinT, perm


def build_program():
    cosT, sinT, perm_np = _host_constants()

    nc = bacc.Bacc(None, target_bir_lowering=False)

    # --- I/O ---------------------------------------------------------
    xb = nc.declare_dram_parameter("xb", [S, D], BF16, isOutput=False)
    wq = nc.declare_dram_parameter("wq", [D, 256], BF16, isOutput=False)
    wk = nc.declare_dram_parameter("wk", [D, 256], BF16, isOutput=False)
    wv = nc.declare_dram_parameter("wv", [D, 260], BF16, isOutput=False)
    bq = nc.declare_dram_parameter("bq", [NP, 128], F32, isOutput=False)
    bk = nc.declare_dram_parameter("bk", [NP, 128], F32, isOutput=False)
    bv = nc.declare_dram_parameter("bv", [1, 260], BF16, isOutput=False)
    ones = nc.declare_dram_parameter("ones", [128, 128], BF16, isOutput=False)
    perm = nc.declare_dram_parameter("perm", [128, 128], BF16, isOutput=False)
    wout = nc.declare_dram_parameter("wout", [D, D], BF16, isOutput=False)
    bout = nc.declare_dram_parameter("bout", [1, D], BF16, isOutput=False)
    out = nc.declare_dram_parameter("out_s", [B, SLC, D], F32, isOutput=True)

    cos_c = nc.inline_tensor(cosT.astype(ml_dtypes.bfloat16), name="cos_c")
    sin_c = nc.inline_tensor(sinT.astype(ml_dtypes.bfloat16), name="sin_c")

    with tile.TileContext(nc) as tc:
        with (
            tc.tile_pool(name="persist", bufs=1) as pp,
            tc.tile_pool(name="dram", bufs=1, space="DRAM") as dp,
        ):
            # --- constants / weights into SBUF ---------------------------
            perm_s = pp.tile([128, 128], BF16)
            nc.sync.dma_start(out=perm_s[:], in_=perm[:])
            cos_s = pp.tile([128, S], BF16)
            sin_s = pp.tile([128, S], BF16)
            nc.sync.dma_start(out=cos_s[:], in_=cos_c[:])
            nc.sync.dma_start(out=sin_s[:], in_=sin_c[:])
            ones_f = pp.tile([128, 128], BF16)
            nc.sync.dma_start(out=ones_f[:], in_=ones[:])

            wq_s = pp.tile([128, 8, 256], BF16)
            wk_s = pp.tile([128, 8, 256], BF16)
            wv_s = pp.tile([128, 8, 260], BF16)
            nc.sync.dma_start(out=wq_s[:], in_=wq.rearrange("(c p) n -> p c n", p=128))
            nc.sync.dma_start(out=wk_s[:], in_=wk.rearrange("(c p) n -> p c n", p=128))
            nc.sync.dma_start(out=wv_s[:], in_=wv.rearrange("(c p) n -> p c n", p=128))
            bq_s = pp.tile([128, NP], F32)
            bk_s = pp.tile([128, NP], F32)
            bv_s = pp.tile([1, 260], BF16)
            for p in range(NP):
                nc.sync.dma_start(out=bq_s[:, p : p + 1], in_=bq[p][:, None])
                nc.sync.dma_start(out=bk_s[:, p : p + 1], in_=bk[p][:, None])
            nc.sync.dma_start(out=bv_s[:], in_=bv[:])

            # persistent activations
            qt = pp.tile([128, NP * S], BF16)   # rotated Q^T, pair-major
            kt = pp.tile([128, NP * S], BF16)   # rotated K^T
            vt = [pp.tile([128, HPC * 65], BF16, name=f"vt{i}") for i in range(NKB)]
            # attnT[p]: [64, 2*S] bf16 — within-pair head h at cols [S*h, S*(h+1))
            attnT = [pp.tile([64, NP * S], BF16, name=f"attnT{p}") for p in range(NP)]

            # DRAM bounce buffers for the per-pair AllToAll (8 ranks; rank r
            # receives every core's attnT columns [SLC*r, SLC*(r+1)) )
            cc_in = [
                dp.tile([NC, 128, SLC], BF16, name=f"cc_in{p}") for p in range(NP)
            ]
            cc_out = [
                dp.tile([NC, 128, SLC], BF16, name=f"cc_out{p}") for p in range(NP)
            ]

            # =============================================================
            # Phase A: x^T, QKV projection, RoPE
            # =============================================================
            with (
                tc.tile_pool(name="xt_pool", bufs=1) as xtp,
                tc.tile_pool(name="qkraw", bufs=3) as rawp,
                tc.tile_pool(name="pj_psum", bufs=3, space="PSUM") as pjp,
                tc.tile_pool(name="rp_psum", bufs=2, space="PSUM") as rpp,
            ):
                # x^T: [128, 8*2048], chunk dc at cols [2048*dc, +2048).
                # bf16 unlocks the X-bar DMA transpose: straight from DRAM,
                # no PE/PSUM involvement.
                xt = xtp.tile([128, 8 * S], BF16)
                for dc in range(8):
                    nc.sync.dma_start(
                        out=xt[:, S * dc : S * dc + S],
                        in_=xb[:, 128 * dc : 128 * dc + 128],
                        transpose=True,
                    )

                # QKV projection + RoPE, chunk-wise.
                # Q^T/K^T: [dq(128 per pair), s] ; rot = raw*cos + (P.T@raw)*sin
                for p in range(NP):
                    for sc in range(NQC):
                        ssl = slice(QC * sc, QC * sc + QC)
                        for w_s, b_s, rot in (
                            (wq_s, bq_s, qt),
                            (wk_s, bk_s, kt),
                        ):
                            ps = pjp.tile([128, 512], F32, tag="pj")
                            for c in range(8):
                                nc.tensor.matmul(
                                    ps[:],
                                    w_s[:, c, 128 * p : 128 * p + 128],
                                    xt[:, S * c + QC * sc : S * c + QC * sc + QC],
                                    start=(c == 0),
                                    stop=(c == 7),
                                )
                            raw = rawp.tile([128, 512], BF16, tag="raw")
                            nc.scalar.activation(
                                raw[:], ps[:], Ident, bias=b_s[:, p : p + 1]
                            )
                            pr = rpp.tile([128, 512], F32, tag="rp")
                            nc.tensor.matmul(
                                pr[:],
                                perm_s[:],
                                raw[:],
                                start=True,
                                stop=True,
                            )
                            dst = rot[:, S * p + QC * sc : S * p + QC * sc + QC]
                            rtmp = rawp.tile([128, 512], BF16, tag="rtmp")
                            nc.vector.tensor_mul(dst, raw[:], cos_s[:, ssl])
                            nc.vector.tensor_mul(rtmp[:], pr[:], sin_s[:, ssl])
                            nc.vector.tensor_add(dst, dst, rtmp[:])

                # V natural [s, 4*65] with a ones column per head slot
                for sb in range(16):
                    ps = pjp.tile([128, 260], F32, tag="pj")
                    for c in range(8):
                        nc.tensor.matmul(
                            ps[:],
                            xt[:, S * c + 128 * sb : S * c + 128 * sb + 128],
                            wv_s[:, c, :],
                            start=(c == 0),
                            stop=False,
                        )
                    nc.tensor.matmul(
                        ps[:],
                        ones_f[0:1, 0:128],
                        bv_s[:],
                        start=False,
                        stop=True,
                    )
                    nc.vector.tensor_copy(vt[sb][:], ps[:])

            # =============================================================
            # Phase B: attention per (pair, q-chunk), heads interleaved
            # =============================================================
            with (
                tc.tile_pool(name="p_pool", bufs=12) as ppool,
                tc.tile_pool(name="recip", bufs=4) as rcp,
                tc.tile_pool(name="sc_psum", bufs=3, space="PSUM") as scp,
                tc.tile_pool(name="av_psum", bufs=3, space="PSUM") as avp,
                tc.tile_pool(name="bc_psum", bufs=2, space="PSUM") as bcp,
            ):
                for p in range(NP):
                    for qc in range(NQC):
                        qsl = slice(QC * qc, QC * qc + QC)
                        nkb_q = 4 * qc + 4
                        av = [
                            avp.tile([128, 512], F32, tag="av", name=f"av{_h}")
                            for _h in range(2)
                        ]
                        for kb in range(nkb_q):
                            pt = []
                            for h in range(2):
                                hsl = slice(64 * h, 64 * h + 64)
                                sc_ps = scp.tile([128, 512], F32, tag="sc")
                                nc.tensor.matmul(
                                    sc_ps[:],
                                    kt[hsl, S * p + KB * kb : S * p + KB * kb + KB],
                                    qt[hsl, S * p + QC * qc : S * p + QC * qc + QC],
                                    start=True,
                                    stop=True,
                                )
                                p_t = ppool.tile([128, 512], BF16, tag="p")
                                nc.scalar.activation(
                                    p_t[:], sc_ps[:], Exp, scale=float(HD**-0.5)
                                )
                                mrel = kb - 4 * qc
                                if mrel >= 0:
                                    # zero where q < k: keep when c - r - 128*mrel >= 0
                                    nc.gpsimd.affine_select(
                                        out=p_t[:],
                                        in_=p_t[:],
                                        compare_op=mybir.AluOpType.is_ge,
                                        fill=0.0,
                                        base=-128 * mrel,
                                        pattern=[[1, 512]],
                                        channel_multiplier=-1,
                                    )
                                pt.append(p_t)
                            for h in range(2):
                                hl = 2 * p + h
                                nc.tensor.matmul(
                                    av[h][0:65, :],
                                    vt[kb][:, 65 * hl : 65 * hl + 65],
                                    pt[h][:],
                                    start=(kb == 0),
                                    stop=(kb == nkb_q - 1),
                                )
                        for h in range(2):
                            rc = rcp.tile([65, 512], BF16, tag="rc")
                            with nc.allow_low_precision(
                                reason="softmax denom reciprocal feeds an "
                                "fp32r matmul; ~19-bit mantissa is plenty"
                            ):
                                nc.vector.reciprocal(rc[64:65, :], av[h][64:65, :])
                            bc = bcp.tile([64, 512], F32, tag="bc")
                            nc.tensor.matmul(
                                bc[:],
                                ones_f[64:65, 0:64],
                                rc[64:65, :],
                                start=True,
                                stop=True,
                            )
                            avs = rcp.tile([64, 512], F32, tag="avs")
                            nc.vector.tensor_copy(avs[:], av[h][0:64, :])
                            nc.vector.tensor_mul(
                                attnT[p][:, S * h + QC * qc : S * h + QC * qc + QC],
                                avs[:],
                                bc[:],
                            )
                    # exchange this pair's attention output (overlaps next pair)
                    for h in range(2):
                        nc.sync.dma_start(
                            out=cc_in[p].rearrange("g p q -> p g q")[
                                64 * h : 64 * h + 64
                            ],
                            in_=attnT[p][
                                :, S * h : S * h + S
                            ].rearrange("p (g q) -> p g q", g=NC),
                        )
                    nc.gpsimd.collective_compute(
                        "AllToAll",
                        mybir.AluOpType.bypass,
                        ins=[cc_in[p].opt()],
                        outs=[cc_out[p].opt()],
                        replica_groups=[[0, 1, 2, 3, 4, 5, 6, 7]],
                    )

            # =============================================================
            # Phase C: gathered attn^T -> out projection for my q-slice
            # =============================================================
            with (
                tc.tile_pool(name="af_pool", bufs=1) as afp,
                tc.tile_pool(name="wo_pool", bufs=1) as wop,
                tc.tile_pool(name="out_sb", bufs=4) as osp,
                tc.tile_pool(name="op_psum", bufs=2, space="PSUM") as opp,
            ):
                wo_s = wop.tile([128, 8, D], BF16)
                nc.sync.dma_start(
                    out=wo_s[:], in_=wout.rearrange("(c p) n -> p c n", p=128)
                )
                bo_s = wop.tile([1, D], BF16)
                nc.sync.dma_start(out=bo_s[:], in_=bout[:])

                # af[b2][k]: head-dim chunk k (rows [128k, 128k+128) of attn
                # for batch b2) over my SLC q-rows.  Source core of chunk k is
                # 4*b2 + k//2, pair k%2.
                af = [
                    [afp.tile([128, SLC], BF16, name=f"af{b2}_{k}") for k in range(8)]
                    for b2 in range(B)
                ]
                for p in range(NP):
                    for src in range(NC):
                        b2, g = src // 4, src % 4
                        nc.sync.dma_start(
                            out=af[b2][2 * g + p][:], in_=cc_out[p][src]
                        )

                for b2 in range(B):
                    for sb in range(SLC // 128):
                        for nc2 in range(2):
                            nsl = slice(512 * nc2, 512 * nc2 + 512)
                            ps = opp.tile([128, 512], F32, tag="op")
                            nc.tensor.matmul(
                                ps[:],
                                ones_f[0:1, 0:128],
                                bo_s[:, nsl],
                                start=True,
                                stop=False,
                            )
                            for k in range(8):
                                nc.tensor.matmul(
                                    ps[:],
                                    af[b2][k][:, 128 * sb : 128 * sb + 128],
                                    wo_s[:, k, nsl],
                                    start=False,
                                    stop=(k == 7),
                                )
                            o_t = osp.tile([128, 512], F32, tag="o")
                            nc.vector.tensor_copy(o_t[:], ps[:])
                            nc.sync.dma_start(
                                out=out[b2, 128 * sb : 128 * sb + 128, nsl],
                                in_=o_t[:],
                            )
    nc.finalize()
    return nc


_PROGRAM = None


def _get_program():
    global _PROGRAM
    if _PROGRAM is None:
        _PROGRAM = build_program()
    return _PROGRAM


def make_in_maps(x, Wqkv, bqkv, Wout, bout):
    x = np.asarray(x, dtype=np.float32)
    Wqkv = np.asarray(Wqkv, dtype=np.float32)
    bqkv = np.asarray(bqkv, dtype=np.float32)
    Wout = np.asarray(Wout, dtype=np.float32)
    bout = np.asarray(bout, dtype=np.float32)

    wout_bf = Wout.astype(ml_dtypes.bfloat16)
    bout_bf = bout.reshape(1, D).astype(ml_dtypes.bfloat16)
    _, _, perm_np = _host_constants()
    ones_np = np.ones((128, 128), dtype=ml_dtypes.bfloat16)
    in_maps = []
    for c in range(8):
        b, g = c // 4, c % 4
        cols = slice(64 * HPC * g, 64 * HPC * (g + 1))  # this core's head dims
        # V weights augmented with a zero column per head slot; the matching
        # bias element is 1.0, so V tiles come out as [v(64) | 1] per head.
        wv_aug = np.zeros((D, 65 * HPC), dtype=np.float32)
        bv_aug = np.zeros((1, 65 * HPC), dtype=np.float32)
        wv_c = Wqkv[:, 2 * D :][:, cols]
        bv_c = bqkv[2 * D :][cols]
        for h in range(HPC):
            wv_aug[:, 65 * h : 65 * h + 64] = wv_c[:, 64 * h : 64 * h + 64]
            bv_aug[0, 65 * h : 65 * h + 64] = bv_c[64 * h : 64 * h + 64]
            bv_aug[0, 65 * h + 64] = 1.0
        in_maps.append(
            {
                "xb": np.ascontiguousarray(x[:, b, :]).astype(ml_dtypes.bfloat16),
                "wq": np.ascontiguousarray(Wqkv[:, 0 * D :][:, cols]).astype(
                    ml_dtypes.bfloat16
                ),
                "wk": np.ascontiguousarray(Wqkv[:, 1 * D :][:, cols]).astype(
                    ml_dtypes.bfloat16
                ),
                "wv": wv_aug.astype(ml_dtypes.bfloat16),
                "bq": np.ascontiguousarray(bqkv[0 * D :][cols].reshape(NP, 128)),
                "bk": np.ascontiguousarray(bqkv[1 * D :][cols].reshape(NP, 128)),
                "bv": bv_aug.astype(ml_dtypes.bfloat16),
                "ones": ones_np,
                "perm": perm_np.astype(ml_dtypes.bfloat16),
                "wout": wout_bf,
                "bout": bout_bf,
            }
        )
    return in_maps


def unshard(results):
    out = np.empty((S, B, D), dtype=np.float32)
    for r in range(8):
        for b2 in range(B):
            out[SLC * r : SLC * (r + 1), b2, :] = results[r]["out_s"][b2]
    return out


def kernel(x, Wqkv, bqkv, Wout, bout, **_kw):
    nc = _get_program()
    in_maps = make_in_maps(x, Wqkv, bqkv, Wout, bout)
    res = run_bass_kernel_spmd(nc, in_maps, list(range(8)))
    return unshard(res.results)
